# revision 17
# baseline (speedup 1.0000x reference)
"""EnhancedDynamicChannelAttention Trainium2 kernel.

Reference computation (B=16, S=2048, C=1024, H=8, HD=128):
    q[b,h,:]   = pref[b,h]*Wq[:,0] + bq
    k          = f @ Wk.T + bk ;  v = f @ Wv.T + bv       (per head slice)
    scores     = softmax_s(q . k)                          [B,H,S]
    ctx[b,h,:] = sum_s scores * v[b,s,h,:]                 [B,H,HD]
    out        = f + broadcast_s(ctx)

Algebraic folding used here (exact up to fp reassociation):
  - softmax is shift invariant  -> the q.bk term and the +qb constant drop.
  - scores[b,h,s] = f[b,s,h,:] . qk[b,h,:]  with  qk = (pref*Wq+bq) @ Wk
  - sum_s attn = 1  ->  ctx = Wv @ (sum_s attn*f[b,s,h,:]) + bv
  So k/v are never materialized; the kernel is memory bound
  (read f once + write out once = 32 MiB per core).

Distribution: pure data parallel over batch, 2 batches per core, 8 cores.

Per-core device program (per batch):
  - DMA in f as 4 "super tiles" [128 part, 4 rows, 1024] (2 MiB each,
    s = st*512 + p*4 + t so each partition holds 4 contiguous rows).
  - DVE: tmp = f * qk_bcast ; segmented reduce -> scores [128, 4, 8]
  - ACT: E = exp(scores)
  - PE : uwf[8,1024] += E_t.T @ f_t  (fp32r, PSUM accum over 16 sub tiles)
         sumE[8,1]  += E_t.T @ ones
  - tail: wf = diag(uwf)/sumE ; ctx_row = wf @ WvT + bv  (tiny)
  - GPSIMD: partition_broadcast(ctx_row) ; f += ctx_bcast (in place)
  - DMA out.
"""

import numpy as np

B, S, C = 16, 2048, 1024
H, HD = 8, 128
N_CORES = 8
BPC = B // N_CORES          # batches per core
ST = 4                      # s-rows per partition in a super tile
P = 128
SUP = S // (P * ST)         # super tiles per batch (4)

_CACHE = {}


def _build_program():
    import concourse.bass as bass
    import concourse.bacc as bacc
    import concourse.tile as tile
    from concourse import mybir

    f32 = mybir.dt.float32

    nc = bacc.Bacc("TRN2", debug=False, num_devices=N_CORES)
    f_in = nc.dram_tensor("features", [BPC, S, C], f32, kind="ExternalInput")
    qk_in = nc.dram_tensor("qkflat", [BPC, C], f32, kind="ExternalInput")
    wvt_in = nc.dram_tensor("wvt", [HD, HD], f32, kind="ExternalInput")
    bvf_in = nc.dram_tensor("bvflat", [1, C], f32, kind="ExternalInput")
    id8_in = nc.dram_tensor("ident8", [8, 8], f32, kind="ExternalInput")
    ones_in = nc.dram_tensor("ones128", [P, 1], f32, kind="ExternalInput")
    out_t = nc.dram_tensor("out", [BPC, S, C], f32, kind="ExternalOutput")

    with tile.TileContext(nc) as tc:
        with (
            tc.tile_pool(name="fpool", bufs=BPC * SUP) as fpool,
            tc.tile_pool(name="tmppool", bufs=1) as tmppool,
            tc.tile_pool(name="spool", bufs=BPC * SUP) as spool,
            tc.tile_pool(name="small", bufs=2) as small,
            tc.tile_pool(name="singles", bufs=1) as singles,
            tc.tile_pool(name="ps1", bufs=1, space="PSUM") as ps1,
            tc.tile_pool(name="ps2", bufs=2, space="PSUM") as ps2,
            tc.tile_pool(name="dscratch", bufs=2, space="DRAM") as dscratch,
        ):
            wvt_sb = singles.tile([HD, HD], f32)
            nc.sync.dma_start(out=wvt_sb, in_=wvt_in[:, :])
            bvf_sb = singles.tile([1, C], f32)
            nc.sync.dma_start(out=bvf_sb, in_=bvf_in[:, :])
            id8_sb = singles.tile([8, 8], f32)
            nc.sync.dma_start(out=id8_sb, in_=id8_in[:, :])
            ones_sb = singles.tile([P, 1], f32)
            nc.sync.dma_start(out=ones_sb, in_=ones_in[:, :])

            for b in range(BPC):
                # qk row for this batch, broadcast down all partitions
                qk_bc = small.tile([P, C], f32, tag="qkbc")
                nc.sync.dma_start(
                    out=qk_bc, in_=qk_in[b : b + 1, :].to_broadcast([P, C])
                )
                qk_bc3 = qk_bc.rearrange("p (o c) -> p o c", o=1).broadcast_to(
                    [P, ST, C]
                )

                uwf1 = ps2.tile([8, 512], f32, tag="uwf1")
                uwf2 = ps2.tile([8, 512], f32, tag="uwf2")
                sumE = ps2.tile([1, 8], f32, tag="sumE", bufs=1)

                fview = f_in[b].rearrange("(st p t) c -> st p t c", p=P, t=ST)
                oview = out_t[b].rearrange("(st p t) c -> st p t c", p=P, t=ST)

                fsups = []
                for st in range(SUP):
                    fsup = fpool.tile([P, ST, C], f32, tag="fsup")
                    nc.sync.dma_start(out=fsup, in_=fview[st])
                    fsups.append(fsup)

                    tmp = tmppool.tile([P, ST, C], f32, tag="tmp")
                    nc.vector.tensor_mul(tmp, fsup, qk_bc3)
                    scores = spool.tile([P, ST, H], f32, tag="scores")
                    nc.vector.reduce_sum(
                        scores,
                        tmp.rearrange("p t (h d) -> p t h d", h=H),
                        axis=mybir.AxisListType.X,
                    )
                    E_sup = spool.tile([P, ST, H], f32, tag="esup")
                    nc.scalar.activation(
                        out=E_sup.rearrange("p t h -> p (t h)"),
                        in_=scores.rearrange("p t h -> p (t h)"),
                        func=mybir.ActivationFunctionType.Exp,
                    )

                    for t in range(ST):
                        first = st == 0 and t == 0
                        last = st == SUP - 1 and t == ST - 1
                        e_sl = E_sup[:, t, :]
                        nc.tensor.matmul(
                            uwf1,
                            e_sl,
                            fsup[:, t, 0:512],
                            start=first,
                            stop=last,
                        )
                        nc.tensor.matmul(
                            uwf2,
                            e_sl,
                            fsup[:, t, 512:1024],
                            start=first,
                            stop=last,
                        )
                        nc.tensor.matmul(
                            sumE, ones_sb, e_sl, start=first, stop=last
                        )

                # ---- tail: ctx_row = (diag(uwf)/sumE) @ WvT + bv ----
                # reciprocal of sumE, broadcast to all partitions via a
                # DRAM round trip (broadcast APs only work from DRAM here)
                sumE_sb = small.tile([1, H], f32, tag="sumesb")
                nc.scalar.copy(out=sumE_sb, in_=sumE)
                sumE_dram = dscratch.tile([1, H], f32, tag="sumedram")
                nc.sync.dma_start(out=sumE_dram, in_=sumE_sb)
                sumE_bc = small.tile([P, H], f32, tag="sumebc")
                nc.sync.dma_start(
                    out=sumE_bc, in_=sumE_dram[0:1, :].to_broadcast([P, H])
                )
                recip_bc = small.tile([P, H], f32, tag="recipbc")
                nc.vector.reciprocal(recip_bc, sumE_bc)

                # uwf [8, 1024] -> SBUF, then per-head PE transpose into
                # a [128, 8*8] tile whose diagonal columns (stride 9) are
                # wfT[d, h] = uwf[h, h*128+d].
                uwf_sb = small.tile([8, C], f32, tag="uwfsb")
                nc.scalar.copy(out=uwf_sb[:, 0:512], in_=uwf1)
                nc.scalar.copy(out=uwf_sb[:, 512:1024], in_=uwf2)
                wfT8_ps = ps1.tile([P, H * H], f32, tag="wft8")
                for h in range(H):
                    nc.tensor.transpose(
                        wfT8_ps[:, h * H : (h + 1) * H],
                        uwf_sb[:, h * HD : (h + 1) * HD],
                        id8_sb,
                    )
                wfT8_sb = small.tile([P, H * H], f32, tag="wft8sb")
                nc.scalar.copy(out=wfT8_sb, in_=wfT8_ps)
                # normalized diagonal: wfTn[:, h] = wfT8_sb[:, 9h] / sumE[h]
                diag_view = bass.AP(
                    tensor=wfT8_sb.tensor,
                    offset=wfT8_sb.offset,
                    ap=[list(wfT8_sb.ap[0]), [H + 1, H]],
                )
                wfTn = small.tile([P, H], f32, tag="wftn")
                nc.vector.tensor_mul(wfTn, diag_view, recip_bc)

                ctx_ps = ps1.tile([1, C], f32, tag="ctxrow")
                for h in range(H):
                    nc.tensor.matmul(
                        ctx_ps[0:1, h * HD : (h + 1) * HD],
                        wfTn[:, h : h + 1],
                        wvt_sb,
                        start=True,
                        stop=True,
                    )
                ctx_row = small.tile([1, C], f32, tag="ctxrowsb")
                nc.vector.tensor_add(ctx_row, ctx_ps, bvf_sb)
                ctx_dram = dscratch.tile([1, C], f32, tag="ctxdram")
                nc.sync.dma_start(out=ctx_dram, in_=ctx_row)
                ctx_bc = small.tile([P, C], f32, tag="ctxbc")
                nc.sync.dma_start(
                    out=ctx_bc, in_=ctx_dram[0:1, :].to_broadcast([P, C])
                )
                ctx_bc3 = ctx_bc.rearrange("p (o c) -> p o c", o=1).broadcast_to(
                    [P, ST, C]
                )

                for st in range(SUP):
                    fsup = fsups[st]
                    nc.gpsimd.tensor_add(fsup, fsup, ctx_bc3)
                    nc.sync.dma_start(out=oview[st], in_=fsup)

    nc.finalize()
    return nc


def _get_program():
    if "nc" not in _CACHE:
        _CACHE["nc"] = _build_program()
    return _CACHE["nc"]


def _prep_in_maps(features, preference, Wq, bq, Wk, Wv, bv):
    f32 = np.float32
    # qk[b,h,:] = (pref[b,h]*Wq[:,0] + bq) @ Wk   -> flat [B, C]
    q = preference[:, :, None] * Wq[:, 0][None, None, :] + bq  # [B,H,HD]
    qk = np.einsum("bhe,ed->bhd", q, Wk)  # [B,H,HD]
    qkflat = np.ascontiguousarray(qk.reshape(B, C), dtype=f32)
    wvt = np.ascontiguousarray(Wv.T, dtype=f32)
    bvflat = np.ascontiguousarray(np.tile(bv, H)[None, :], dtype=f32)
    id8 = np.eye(8, dtype=f32)
    ones128 = np.ones([P, 1], dtype=f32)

    in_maps = []
    for i in range(N_CORES):
        sl = slice(i * BPC, (i + 1) * BPC)
        in_maps.append(
            {
                "features": np.ascontiguousarray(features[sl], dtype=f32),
                "qkflat": qkflat[sl],
                "wvt": wvt,
                "bvflat": bvflat,
                "ident8": id8,
                "ones128": ones128,
            }
        )
    return in_maps


def kernel(features, preference, Wq, bq, Wk, bk, Wv, bv, **_ignored):
    features = np.asarray(features, dtype=np.float32)
    preference = np.asarray(preference, dtype=np.float32)
    Wq = np.asarray(Wq, dtype=np.float32)
    bq = np.asarray(bq, dtype=np.float32)
    Wk = np.asarray(Wk, dtype=np.float32)
    Wv = np.asarray(Wv, dtype=np.float32)
    bv = np.asarray(bv, dtype=np.float32)

    from concourse.bass_utils import run_bass_kernel_spmd

    nc = _get_program()
    in_maps = _prep_in_maps(features, preference, Wq, bq, Wk, Wv, bv)
    res = run_bass_kernel_spmd(nc, in_maps, core_ids=list(range(N_CORES)))
    out = np.concatenate([r["out"] for r in res.results], axis=0)
    return out.astype(np.float32)


# revision 19
# speedup vs baseline: 1.1037x; 1.1037x over previous
"""EnhancedDynamicChannelAttention Trainium2 kernel.

Reference computation (B=16, S=2048, C=1024, H=8, HD=128):
    q[b,h,:]   = pref[b,h]*Wq[:,0] + bq
    k          = f @ Wk.T + bk ;  v = f @ Wv.T + bv       (per head slice)
    scores     = softmax_s(q . k)                          [B,H,S]
    ctx[b,h,:] = sum_s scores * v[b,s,h,:]                 [B,H,HD]
    out        = f + broadcast_s(ctx)

Algebraic folding used here (exact up to fp reassociation):
  - softmax is shift invariant  -> the q.bk term drops entirely.
  - scores[b,h,s] = f[b,s,h,:] . qk[b,h,:]  with  qk = (pref*Wq+bq) @ Wk
  - sum_s attn = 1  ->  ctx = Wv @ (sum_s attn*f[b,s,h,:]) + bv
  So k/v are never materialized; the kernel is memory bound
  (read f once + write out once = 32 MiB per core).

Distribution: pure data parallel over batch, 2 batches per core, 8 cores.

Per-core device program (per batch b, f kept resident in SBUF):
  - DMA in f as 4 x 2MiB super tiles into one [128, 16, 1024] tile
    (s = st*512 + p*4 + t: partition p holds 4 contiguous rows per st).
  - DVE  : tmp = f * qk_bcast ; segmented reduce -> scores [128, 4, 8]
  - ACT  : E = exp(scores)               (no max-sub needed; |scores|<~30)
  - PE   : uwf[8,1024] += E_t.T @ f_t ; sumE[8,1] += E_t.T @ ones
           (fp32, PSUM accumulation over the 16 sub tiles)
  - tail : uwf /= sumE (row scale) ; per-head PE transpose -> wfT ;
           ctx_row[1,1024] = wfT_h.T @ WvT + bv ; broadcast via DRAM.
  - adds : f += ctx_bcast in place, split DVE/GPSIMD, then DMA out.
"""

import numpy as np

B, S, C = 16, 2048, 1024
H, HD = 8, 128
N_CORES = 8
BPC = B // N_CORES          # batches per core
ST = 4                      # s-rows per partition in a super tile
P = 128
SUP = S // (P * ST)         # super tiles per batch (4)
NT = S // P                 # sub tiles per batch (16)

_CACHE = {}


def _build_program():
    import concourse.bass as bass
    import concourse.bacc as bacc
    import concourse.tile as tile
    from concourse import mybir

    f32 = mybir.dt.float32

    nc = bacc.Bacc("TRN2", debug=False, num_devices=N_CORES)
    f_in = nc.dram_tensor("features", [BPC, S, C], f32, kind="ExternalInput")
    qk_in = nc.dram_tensor("qkflat", [BPC, C], f32, kind="ExternalInput")
    wvt_in = nc.dram_tensor("wvt", [HD, HD], f32, kind="ExternalInput")
    bvf_in = nc.dram_tensor("bvflat", [1, C], f32, kind="ExternalInput")
    id8_in = nc.dram_tensor("ident8", [8, 8], f32, kind="ExternalInput")
    ones_in = nc.dram_tensor("ones128", [P, 1], f32, kind="ExternalInput")
    out_t = nc.dram_tensor("out", [BPC, S, C], f32, kind="ExternalOutput")

    with tile.TileContext(nc) as tc:
        with (
            tc.tile_pool(name="fpool", bufs=BPC) as fpool,
            tc.tile_pool(name="tmppool", bufs=2) as tmppool,
            tc.tile_pool(name="spool", bufs=2 * SUP) as spool,
            tc.tile_pool(name="small", bufs=2) as small,
            tc.tile_pool(name="singles", bufs=1) as singles,
            tc.tile_pool(name="ps1", bufs=1, space="PSUM") as ps1,
            tc.tile_pool(name="ps2", bufs=2, space="PSUM") as ps2,
            tc.tile_pool(name="dscratch", bufs=2, space="DRAM") as dscratch,
        ):
            wvt_sb = singles.tile([HD, HD], f32)
            nc.sync.dma_start(out=wvt_sb, in_=wvt_in[:, :])
            bvf_sb = singles.tile([1, C], f32)
            nc.sync.dma_start(out=bvf_sb, in_=bvf_in[:, :])
            id8_sb = singles.tile([8, 8], f32)
            nc.sync.dma_start(out=id8_sb, in_=id8_in[:, :])
            ones_sb = singles.tile([P, 1], f32)
            nc.sync.dma_start(out=ones_sb, in_=ones_in[:, :])

            for b in range(BPC):
                # qk row for this batch, broadcast down all partitions
                qk_bc = small.tile([P, C], f32, tag="qkbc")
                nc.sync.dma_start(
                    out=qk_bc, in_=qk_in[b : b + 1, :].to_broadcast([P, C])
                )
                qk_bc3 = qk_bc.rearrange("p (o c) -> p o c", o=1).broadcast_to(
                    [P, ST, C]
                )

                uwf1 = ps2.tile([8, 512], f32, tag="uwf1")
                uwf2 = ps2.tile([8, 512], f32, tag="uwf2")
                sumE = ps2.tile([8, 1], f32, tag="sumE", bufs=1)

                fview = f_in[b].rearrange("(st p t) c -> st p t c", p=P, t=ST)
                oview = out_t[b].rearrange("(st p t) c -> st p t c", p=P, t=ST)

                fb = fpool.tile([P, NT, C], f32, tag="fb")
                for st in range(SUP):
                    fsl = fb[:, st * ST : (st + 1) * ST, :]
                    nc.sync.dma_start(out=fsl, in_=fview[st])

                    tmp = tmppool.tile([P, ST, C], f32, tag="tmp")
                    nc.vector.tensor_mul(tmp, fsl, qk_bc3)
                    scores = spool.tile([P, ST, H], f32, tag="scores")
                    nc.vector.reduce_sum(
                        scores,
                        tmp.rearrange("p t (h d) -> p t h d", h=H),
                        axis=mybir.AxisListType.X,
                    )
                    E_sup = spool.tile([P, ST, H], f32, tag="esup")
                    nc.scalar.activation(
                        out=E_sup.rearrange("p t h -> p (t h)"),
                        in_=scores.rearrange("p t h -> p (t h)"),
                        func=mybir.ActivationFunctionType.Exp,
                    )

                    for t in range(ST):
                        first = st == 0 and t == 0
                        last = st == SUP - 1 and t == ST - 1
                        e_sl = E_sup[:, t, :]
                        f_sl = fb[:, st * ST + t, :]
                        nc.tensor.matmul(
                            uwf1, e_sl, f_sl[:, 0:512], start=first, stop=last
                        )
                        nc.tensor.matmul(
                            uwf2, e_sl, f_sl[:, 512:1024], start=first, stop=last
                        )
                        nc.tensor.matmul(
                            sumE, e_sl, ones_sb, start=first, stop=last
                        )

                # ---- tail: ctx_row = (diag(uwf)/sumE) @ WvT + bv ----
                recip = small.tile([8, 1], f32, tag="recip")
                nc.vector.reciprocal(recip, sumE)
                # uwf -> SBUF, normalized rows: uwf[h,:] / sumE[h]
                uwf_sb = small.tile([8, C], f32, tag="uwfsb")
                nc.scalar.copy(out=uwf_sb[:, 0:512], in_=uwf1)
                nc.scalar.copy(out=uwf_sb[:, 512:1024], in_=uwf2)
                nc.vector.tensor_scalar_mul(uwf_sb, uwf_sb, recip)
                # per-head PE transpose into [128, 8*8]; diagonal columns
                # (stride 9) hold wfT[d, h] = uwf[h, h*128+d] / sumE[h]
                wfT8_ps = ps1.tile([P, H * H], f32, tag="wft8")
                for h in range(H):
                    nc.tensor.transpose(
                        wfT8_ps[:, h * H : (h + 1) * H],
                        uwf_sb[:, h * HD : (h + 1) * HD],
                        id8_sb,
                    )
                wfT8_sb = small.tile([P, H * H], f32, tag="wft8sb")
                nc.scalar.copy(out=wfT8_sb, in_=wfT8_ps)

                ctx_ps = ps1.tile([1, C], f32, tag="ctxrow")
                for h in range(H):
                    nc.tensor.matmul(
                        ctx_ps[0:1, h * HD : (h + 1) * HD],
                        wfT8_sb[:, h * (H + 1) : h * (H + 1) + 1],
                        wvt_sb,
                        start=True,
                        stop=True,
                    )
                ctx_row = small.tile([1, C], f32, tag="ctxrowsb")
                nc.vector.tensor_add(ctx_row, ctx_ps, bvf_sb)
                ctx_dram = dscratch.tile([1, C], f32, tag="ctxdram")
                nc.sync.dma_start(out=ctx_dram, in_=ctx_row)
                ctx_bc = small.tile([P, C], f32, tag="ctxbc")
                nc.sync.dma_start(
                    out=ctx_bc, in_=ctx_dram[0:1, :].to_broadcast([P, C])
                )
                ctx_bc3 = ctx_bc.rearrange("p (o c) -> p o c", o=1).broadcast_to(
                    [P, ST, C]
                )

                # residual adds split across DVE / GPSIMD, DMA out per tile
                for st in range(SUP):
                    fsl = fb[:, st * ST : (st + 1) * ST, :]
                    if st % 2 == 0:
                        nc.gpsimd.tensor_add(fsl, fsl, ctx_bc3)
                    else:
                        nc.vector.tensor_add(fsl, fsl, ctx_bc3)
                    nc.sync.dma_start(out=oview[st], in_=fsl)

    nc.finalize()
    return nc


def _get_program():
    if "nc" not in _CACHE:
        _CACHE["nc"] = _build_program()
    return _CACHE["nc"]


def _prep_in_maps(features, preference, Wq, bq, Wk, Wv, bv):
    f32 = np.float32
    # qk[b,h,:] = (pref[b,h]*Wq[:,0] + bq) @ Wk   -> flat [B, C]
    q = preference[:, :, None] * Wq[:, 0][None, None, :] + bq  # [B,H,HD]
    qk = np.einsum("bhe,ed->bhd", q, Wk)  # [B,H,HD]
    qkflat = np.ascontiguousarray(qk.reshape(B, C), dtype=f32)
    wvt = np.ascontiguousarray(Wv.T, dtype=f32)
    bvflat = np.ascontiguousarray(np.tile(bv, H)[None, :], dtype=f32)
    id8 = np.eye(8, dtype=f32)
    ones128 = np.ones([P, 1], dtype=f32)

    in_maps = []
    for i in range(N_CORES):
        sl = slice(i * BPC, (i + 1) * BPC)
        in_maps.append(
            {
                "features": np.ascontiguousarray(features[sl], dtype=f32),
                "qkflat": qkflat[sl],
                "wvt": wvt,
                "bvflat": bvflat,
                "ident8": id8,
                "ones128": ones128,
            }
        )
    return in_maps


def kernel(features, preference, Wq, bq, Wk, bk, Wv, bv, **_ignored):
    features = np.asarray(features, dtype=np.float32)
    preference = np.asarray(preference, dtype=np.float32)
    Wq = np.asarray(Wq, dtype=np.float32)
    bq = np.asarray(bq, dtype=np.float32)
    Wk = np.asarray(Wk, dtype=np.float32)
    Wv = np.asarray(Wv, dtype=np.float32)
    bv = np.asarray(bv, dtype=np.float32)

    from concourse.bass_utils import run_bass_kernel_spmd

    nc = _get_program()
    in_maps = _prep_in_maps(features, preference, Wq, bq, Wk, Wv, bv)
    res = run_bass_kernel_spmd(nc, in_maps, core_ids=list(range(N_CORES)))
    out = np.concatenate([r["out"] for r in res.results], axis=0)
    return out.astype(np.float32)
